# revision 1
# baseline (speedup 1.0000x reference)
"""Causal GQA self-attention (B=2, S=2048, D=2048, H=16, KV=4) on 8 TRN2 cores.

Sharding: core = (b, g) with b = batch (2) x g = kv-head group (4).
Each core computes 4 q-heads / 1 kv-head for one batch and a partial
projection output [S, D]; host sums the 4 group partials per batch.

Per-core pipeline (all matmuls in float32r = full-rate, ~1e-4 precision):
  1. QKV: q/k/v in natural [s, d] layout (lhsT = xT blocks, rhs = W tiles).
     RMS-norm scale via ACT Square+accum; rope+norm fused into DVE
     scalar_tensor_tensor ops; PE-transpose q/k into [hd, S] layout.
  2. Attention per (q-slice t of 512, head h): scoresT[k,q] blocks via
     matmul(lhsT=kT block, rhs=qT slice); exp on ACT (no max subtraction:
     |score| <= gain*sqrt(hd)); causal via host-provided mask tiles;
     PV via matmul(lhsT=v block, rhs=probsT) -> outT[hd, q]; row sums via
     ones-vector matmul; normalize with reciprocal + partition_broadcast.
  3. Proj: out[s, dout] partial = sum_h yT[h].T @ WprojT[h] accumulated in
     PSUM, written to DRAM.
"""
import os
import sys

if '/opt/trn_rl_repo' not in sys.path:
    sys.path.insert(0, '/opt/trn_rl_repo')

import numpy as np

B, S, D = 2, 2048, 2048
NH_TOT, NKV_TOT, HD = 16, 4, 128
NH = 4                 # q heads per core
NT = S // 128          # 16 s-tiles
NC_ = D // 128         # 16 c-tiles
T = 4                  # q-slices of 512
SM = 1.0 / np.sqrt(HD)
EPS = float(np.finfo(np.float32).eps)
ROPE_BASE = 10000.0

_PROG = None


def _build_program():
    import concourse.bass as bass
    import concourse.mybir as mybir
    import concourse.tile as tile
    from concourse import bacc
    from concourse.alu_op_type import AluOpType

    F32 = mybir.dt.float32
    F32R = mybir.dt.float32r
    AF = mybir.ActivationFunctionType

    nc = bacc.Bacc("TRN2", target_bir_lowering=False, debug=False)

    XT = nc.dram_tensor("XT", [D, S], F32R, kind="ExternalInput")          # x[b].T
    WQ = nc.dram_tensor("WQ", [128, NC_, 512], F32R, kind="ExternalInput")  # Wq_g.T tiled [c_p, ci, dq]
    WKV = nc.dram_tensor("WKV", [128, NC_, 256], F32R, kind="ExternalInput")
    WP = nc.dram_tensor("WP", [128, NH, D], F32R, kind="ExternalInput")     # [c_in_head, h, dout]
    COS2 = nc.dram_tensor("COS2", [128, NT, HD], F32, kind="ExternalInput")
    SIN2 = nc.dram_tensor("SIN2", [128, NT, HD], F32, kind="ExternalInput")
    GSM = nc.dram_tensor("GSM", [1, NH], F32, kind="ExternalInput")         # gain*sm per head
    IDENT = nc.dram_tensor("IDENT", [128, 128], F32R, kind="ExternalInput")
    ONES = nc.dram_tensor("ONES", [128, 1], F32R, kind="ExternalInput")
    MASKS = nc.dram_tensor("MASKS", [4, 128, 512], F32R, kind="ExternalInput")
    Y = nc.dram_tensor("Y", [S, D], F32, kind="ExternalOutput")

    with tile.TileContext(nc) as tc:
        with (
            tc.tile_pool(name="const", bufs=1) as const,
            tc.tile_pool(name="w", bufs=4) as wpool,
            tc.tile_pool(name="stream", bufs=4) as stream,
            tc.tile_pool(name="small", bufs=3) as small,
            tc.tile_pool(name="norm", bufs=2) as normp,
            tc.tile_pool(name="rope", bufs=3) as ropep,
            tc.tile_pool(name="big", bufs=1) as big,
            tc.tile_pool(name="yt", bufs=2) as ytp,
            tc.tile_pool(name="probs", bufs=4) as probsp,
            tc.tile_pool(name="outsb", bufs=2) as outsb,
            tc.tile_pool(name="psA", bufs=2, space="PSUM") as psA,
            tc.tile_pool(name="psB", bufs=2, space="PSUM") as psB,
            tc.tile_pool(name="psS", bufs=2, space="PSUM") as psS,
        ):
            # small consts first (cheap), then weights in 4 chunks so the
            # first matmuls start after ~1.5MB of weight DMA, not 6MB
            ident = const.tile([128, 128], F32R)
            nc.sync.dma_start(ident[:], IDENT[:])
            ones = const.tile([128, 1], F32R)
            nc.sync.dma_start(ones[:], ONES[:])
            gsm = const.tile([1, NH], F32)
            nc.sync.dma_start(gsm[:], GSM[:])
            gsm_bc = const.tile([128, NH], F32)
            nc.gpsimd.partition_broadcast(gsm_bc[:], gsm[:])

            wqkv = []
            for c4 in range(4):
                wt = wpool.tile([128, 4, 768], F32R, tag="w")
                nc.scalar.dma_start(wt[:, :, 0:512], WQ[:, 4 * c4:4 * c4 + 4, :])
                nc.scalar.dma_start(wt[:, :, 512:768], WKV[:, 4 * c4:4 * c4 + 4, :])
                wqkv.append(wt)

            qT = big.tile([128, NH, S], F32R)
            kT = big.tile([128, S], F32R)
            v_nat = big.tile([128, NT, HD], F32R)

            cos2 = None
            sin2 = None
            masks = const.tile([128, 4, 512], F32R)

            # ---------------- phase 1: QKV + rms-norm + rope + transpose
            for si in range(NT):
                xs = stream.tile([128, NC_, 128], F32R, tag="xs")
                nc.sync.dma_start(
                    xs[:], XT[:, si * 128:(si + 1) * 128].rearrange("(a p) s -> p a s", p=128))
                q_ps = psA.tile([128, 512], F32, tag="A")
                kv_ps = psB.tile([128, 256], F32, tag="B")
                for ci in range(NC_):
                    nc.tensor.matmul(q_ps[:], xs[:, ci, :], wqkv[ci // 4][:, ci % 4, 0:512],
                                     start=(ci == 0), stop=(ci == NC_ - 1))
                    nc.tensor.matmul(kv_ps[:], xs[:, ci, :], wqkv[ci // 4][:, ci % 4, 512:768],
                                     start=(ci == 0), stop=(ci == NC_ - 1))
                if si == 0:
                    # cos/sin share the xs streaming slots; needed once rope
                    # starts, so posted behind the first x s-tile
                    cos2 = stream.tile([128, NT, HD], F32, tag="xs")
                    nc.sync.dma_start(cos2[:], COS2[:])
                    sin2 = stream.tile([128, NT, HD], F32, tag="xs")
                    nc.sync.dma_start(sin2[:], SIN2[:])
                    # masks are first needed by attention; scalar queue
                    nc.scalar.dma_start(masks[:], MASKS[:].transpose([1, 0, 2]))

                # sum of squares per head (q: 4 heads, k: 1)
                scr = small.tile([128, 128], F32, tag="scr")
                ssq = small.tile([128, 8], F32, tag="ssq")
                for h in range(NH):
                    nc.scalar.activation(scr[:], q_ps[:, h * 128:(h + 1) * 128],
                                         AF.Square, accum_out=ssq[:, h:h + 1])
                nc.scalar.activation(scr[:], kv_ps[:, 0:128], AF.Square,
                                     accum_out=ssq[:, 4:5])
                mn = small.tile([128, 8], F32, tag="mn")
                nc.vector.tensor_scalar(mn[:, 0:5], ssq[:, 0:5], 1.0 / HD, EPS,
                                        AluOpType.mult, AluOpType.add)
                rt = small.tile([128, 8], F32, tag="rt")
                nc.scalar.sqrt(rt[:, 0:5], mn[:, 0:5])
                rn = small.tile([128, 8], F32, tag="rn")
                nc.vector.reciprocal(rn[:, 0:5], rt[:, 0:5])
                qsc = small.tile([128, 4], F32, tag="qsc")
                nc.vector.tensor_tensor(qsc[:], rn[:, 0:4], gsm_bc[:], AluOpType.mult)

                # rope + norm-scale fused; then PE transpose into T layout
                for h in range(NH + 1):
                    if h < NH:
                        raw = q_ps[:, h * 128:(h + 1) * 128]
                        sc_ap = qsc[:, h:h + 1]
                    else:
                        raw = kv_ps[:, 0:128]
                        sc_ap = rn[:, 4:5]
                    tcs = ropep.tile([128, 128], F32, tag="tcs")
                    tsn = ropep.tile([128, 128], F32, tag="tsn")
                    nc.vector.scalar_tensor_tensor(
                        tcs[:], raw, sc_ap, cos2[:, si, :], AluOpType.mult, AluOpType.mult)
                    nc.vector.scalar_tensor_tensor(
                        tsn[:, 0:64], raw[:, 64:128], sc_ap, sin2[:, si, 0:64],
                        AluOpType.mult, AluOpType.mult)
                    nc.vector.scalar_tensor_tensor(
                        tsn[:, 64:128], raw[:, 0:64], sc_ap, sin2[:, si, 64:128],
                        AluOpType.mult, AluOpType.mult)
                    nat = ropep.tile([128, 128], F32R, tag="nat")
                    nc.gpsimd.tensor_tensor(nat[:], tcs[:], tsn[:], AluOpType.add)
                    tp = psS.tile([128, 128], F32R, tag="S")
                    nc.tensor.transpose(tp[:], nat[:], ident[:])
                    if h < NH:
                        nc.scalar.copy(qT[:, h, si * 128:(si + 1) * 128], tp[:])
                    else:
                        nc.scalar.copy(kT[:, si * 128:(si + 1) * 128], tp[:])

                # v: plain copy out of psum
                nc.scalar.copy(v_nat[:, si, :], kv_ps[:, 128:256])

            # proj weights into the freed w slots (4 dout-quarters)
            wp = []
            for dq in range(4):
                wt = wpool.tile([128, NH, 512], F32R, tag="w")
                nc.scalar.dma_start(wt[:], WP[:, :, dq * 512:(dq + 1) * 512])
                wp.append(wt)

            # ---------------- phase 2: attention, two heads interleaved
            pending_epilogue = [None]

            def flush_epilogue():
                if pending_epilogue[0] is not None:
                    pending_epilogue[0]()
                    pending_epilogue[0] = None

            for t in range(T):
                yt_t = ytp.tile([128, NH, 512], F32R, tag="yt")
                nblk = 4 * t + 4
                for hp in (0, 2):
                    o_ps = {}
                    rs_ps = {}
                    for h in (hp, hp + 1):
                        o_ps[h] = psA.tile([128, 512], F32, tag="A", name=f"o_ps_{t}_{h}")
                        rs_ps[h] = psB.tile([1, 512], F32, tag="B", name=f"rs_ps_{t}_{h}")
                    for j in range(nblk):
                        sc = psS.tile([128, 1024], F32, tag="S", name=f"sc_{t}_{hp}_{j}")
                        for u, h in enumerate((hp, hp + 1)):
                            nc.tensor.matmul(
                                sc[:, u * 512:(u + 1) * 512],
                                kT[:, j * 128:(j + 1) * 128],
                                qT[:, h, t * 512:(t + 1) * 512],
                                start=True, stop=True)
                        prb = probsp.tile([128, 1024], F32R, tag="probs", name=f"prb_{t}_{hp}_{j}")
                        off = j - 4 * t
                        # exp for both heads in one ACT call
                        nc.scalar.activation(prb[:], sc[:], AF.Exp)
                        if off >= 0:
                            w_ = (off + 1) * 128
                            for u in range(2):
                                nc.vector.tensor_tensor(
                                    prb[:, u * 512:u * 512 + w_],
                                    prb[:, u * 512:u * 512 + w_],
                                    masks[:, off, 0:w_], AluOpType.mult)
                        for u, h in enumerate((hp, hp + 1)):
                            nc.tensor.matmul(
                                o_ps[h][:], v_nat[:, j, :], prb[:, u * 512:(u + 1) * 512],
                                start=(j == 0), stop=(j == nblk - 1), skip_group_check=True)
                            nc.tensor.matmul(
                                rs_ps[h][:], ones[:], prb[:, u * 512:(u + 1) * 512],
                                start=(j == 0), stop=(j == nblk - 1), skip_group_check=True)
                        if j == 0:
                            flush_epilogue()

                    # evict psum (frees o/rs slots), normalize off the PE
                    # critical path; emission deferred into the next unit
                    def make_epilogue(o_ps=o_ps, rs_ps=rs_ps, yt_t=yt_t, hp=hp):
                        def ep():
                            for h in (hp, hp + 1):
                                nc.vector.tensor_copy(yt_t[:, h, :], o_ps[h][:])
                                rs_sb = normp.tile([1, 512], F32, tag="rssb")
                                nc.vector.tensor_copy(rs_sb[:], rs_ps[h][:])
                                rs_bc = normp.tile([128, 512], F32, tag="rsbc")
                                nc.gpsimd.partition_broadcast(rs_bc[:], rs_sb[:])
                                rcp_bc = normp.tile([128, 512], F32, tag="rcpbc")
                                nc.vector.reciprocal(rcp_bc[:], rs_bc[:])
                                nc.vector.tensor_tensor(
                                    yt_t[:, h, :], yt_t[:, h, :], rcp_bc[:], AluOpType.mult)
                        return ep
                    pending_epilogue[0] = make_epilogue()

                # ---------------- phase 3 (per t): projection for s-tiles 4t..4t+3
                flush_epilogue()
                for si in range(4 * t, 4 * t + 4):
                    sl = si - 4 * t
                    for dtp in range(2):
                        pj0 = psB.tile([128, 512], F32, tag="B")
                        pj1 = psB.tile([128, 512], F32, tag="B")
                        for h in range(NH):
                            lhs = yt_t[:, h, sl * 128:(sl + 1) * 128]
                            nc.tensor.matmul(pj0[:], lhs, wp[2 * dtp][:, h, :],
                                             start=(h == 0), stop=(h == NH - 1),
                                             skip_group_check=True)
                            nc.tensor.matmul(pj1[:], lhs, wp[2 * dtp + 1][:, h, :],
                                             start=(h == 0), stop=(h == NH - 1),
                                             skip_group_check=True)
                        for k_, pj in enumerate((pj0, pj1)):
                            ev = outsb.tile([128, 512], F32, tag="ev")
                            nc.vector.tensor_copy(ev[:], pj[:])
                            nc.sync.dma_start(
                                Y[si * 128:(si + 1) * 128,
                                  dtp * 1024 + k_ * 512:dtp * 1024 + (k_ + 1) * 512], ev[:])

    nc.compile()
    return nc


def _host_inputs(x, Wq, Wk, Wv, Wproj, q_gain):
    x = np.asarray(x, dtype=np.float32)
    Wq = np.asarray(Wq, dtype=np.float32)
    Wk = np.asarray(Wk, dtype=np.float32)
    Wv = np.asarray(Wv, dtype=np.float32)
    Wproj = np.asarray(Wproj, dtype=np.float32)
    q_gain = np.asarray(q_gain, dtype=np.float32)

    inv = (1.0 / ROPE_BASE ** (np.arange(0, HD, 2, dtype=np.float32) / HD)).astype(np.float32)
    ang = np.outer(np.arange(S, dtype=np.float32), inv)
    cos = np.cos(ang).astype(np.float32)
    sin = np.sin(ang).astype(np.float32)
    cos2 = np.concatenate([cos, cos], 1).reshape(NT, 128, HD).transpose(1, 0, 2).copy()
    sin2 = np.concatenate([sin, -sin], 1).reshape(NT, 128, HD).transpose(1, 0, 2).copy()

    qq = np.arange(512)[None, :]
    kk = np.arange(128)[:, None]
    masks = np.stack([(kk <= qq - off * 128).astype(np.float32) for off in range(4)])
    ident = np.eye(128, dtype=np.float32)
    ones = np.ones((128, 1), dtype=np.float32)

    in_maps = []
    for cid in range(8):
        b, g = cid // 4, cid % 4
        wq = Wq[g * 512:(g + 1) * 512, :].T            # [D, 512]
        wk = Wk[g * 128:(g + 1) * 128, :].T            # [D, 128]
        wv = Wv[g * 128:(g + 1) * 128, :].T
        wkv = np.concatenate([wk, wv], 1)              # [D, 256]
        wp = Wproj[:, g * 512:(g + 1) * 512].T         # [512, D] (c_local, dout)
        in_maps.append({
            "XT": np.ascontiguousarray(x[b].T),
            "WQ": np.ascontiguousarray(wq.reshape(NC_, 128, 512).transpose(1, 0, 2)),
            "WKV": np.ascontiguousarray(wkv.reshape(NC_, 128, 256).transpose(1, 0, 2)),
            "WP": np.ascontiguousarray(wp.reshape(NH, 128, D).transpose(1, 0, 2)),
            "COS2": cos2, "SIN2": sin2,
            "GSM": (q_gain[g * 4:(g + 1) * 4] * SM).reshape(1, NH).astype(np.float32),
            "IDENT": ident, "ONES": ones, "MASKS": masks,
        })
    return in_maps


def _get_prog():
    global _PROG
    if _PROG is None:
        _PROG = _build_program()
    return _PROG


def kernel(x, Wq, Wk, Wv, Wproj, q_gain, _trace=False, _tmpdir=None):
    from concourse.bass_utils import run_bass_kernel_spmd
    nc = _get_prog()
    in_maps = _host_inputs(x, Wq, Wk, Wv, Wproj, q_gain)
    kwargs = {}
    if _tmpdir is not None:
        os.makedirs(_tmpdir, exist_ok=True)
        kwargs["tmpdir"] = _tmpdir
    res = run_bass_kernel_spmd(nc, in_maps, list(range(8)), trace=_trace, **kwargs)
    y = np.empty((B, S, D), dtype=np.float32)
    for b in range(B):
        acc = res.results[4 * b]["Y"].astype(np.float32).copy()
        for g in range(1, 4):
            acc += res.results[4 * b + g]["Y"]
        y[b] = acc
    if _trace:
        kernel._last_result = res
    return y

